# revision 35
# baseline (speedup 1.0000x reference)
"""Trainium2 Bass kernel for nn_F1_67379446940315 (histogram_binning F1 metric).

Computes: pred = argmax(y_pred, axis=1); conf = scatter-add confusion matrix;
then the (quirky, faithful-to-reference) per-class F1 reduction to a scalar.

Strategy (8 NeuronCores, data-parallel over N; ~180us/iter vs ~103us bf16 DMA
floor per core at ~330 GB/s):
  - host converts y_pred to bf16 (halves HBM traffic; bf16 argmax rounding
    ties shift the conf counts by ~0.4% of rows -> F1 rel err ~1.7e-3,
    tolerance 2e-2)
  - each core streams its shard [131072, 128] bf16 in 1 MiB chunks
    [128 partitions, 32 rows x 128 classes]
  - per-row max via a 3-op DVE cascade: two pairwise-max tensor_tensors
    (2x bf16 mode) + one small 1x tensor_reduce -- ~35% cheaper than a
    single full-width 1x reduce
  - pred one-hot mask, split across engines (all are near-saturated):
    - 60% of chunks on DVE via ONE 4D tensor_tensor is_equal against a
      PE-materialized replicated row-max ("pemxr"): PE transposes mx,
      a selector matmul broadcasts it to [128, rpp*32] PSUM, ACT copies
      it to SBUF bf16; the group-broadcast TT then runs without the
      16 drain-separated per-row tensor_scalars it replaces
    - 40% on ACT (ScalarE): Sign(rowmax - x) per row slice = an INVERTED
      {0,1} mask into a second PSUM bank; the host undoes the inversion
      exactly (integer algebra)
  - true one-hot T built by GPSIMD local_scatter from host-precomputed
    int16 indices (r%8)*128 + y_true -- zero DVE/ACT cost
  - PE matmul accumulation: conf_psum[bank] += T_r^T @ S_r (contraction
    over the 128 rows on partitions), 32 matmuls per chunk, emitted with a
    one-chunk delay so the in-order PE stream interleaves the next chunk's
    transpose/selector matmuls instead of stalling on the mask
  - per-core [128, 256] (bank A | bank B) f32 DMA'd out; host reconstructs
    conf = A + (cntB - B), sums the 8 partials, and does the tiny F1
    reduction (negligible work, replicated per the sharding hint).

Engine budget per 1 MiB chunk (~5.6us at 180us/iter, DMA floor 3.2us):
DVE cascade 2.85 + mask 0.6*4.4 = 5.5; ACT 0.4*12.7 + copies 0.77 = 5.9;
PE 32 matmuls + transpose/select ~4; Pool 2 local_scatters ~0.5.
"""

import numpy as np
import ml_dtypes
from contextlib import ExitStack

import concourse.bass as bass
import concourse.bacc as bacc
import concourse.tile as tile
from concourse import mybir
from concourse import bass_utils

N_TOTAL = 1048576
C = 128
N_CORES = 8
SHARD = N_TOTAL // N_CORES  # 131072
EPS = np.float32(1e-12)

BF16 = mybir.dt.bfloat16
F32 = mybir.dt.float32


def build_conf_kernel(ctx, tc, conf_out, yp, yt, iota_ap, n_rows, rpp=8, reps=1,
                      stages=("dma", "reduce", "mask", "onehot", "matmul"),
                      yt_dtype=F32, onehot_mode="ts", mask_mode="dve",
                      dma_split=1, act_frac=0.5, gps_frac=0.0, bufs_x=3, bufs_st=3,
                      act_slices=8, loop_hints=False, dma_alt=False, act_pairs=False,
                      xdt=F32, reduce_mode="direct", dve_mask="tt",
                      ident_ap=None, sel_ap=None, unroll=1, mxt_dma=False):
    """Emit the per-core confusion-matrix kernel.

    conf_out: DRAM [128,256] f32 output AP (cols 0:128 = bank A is_equal
              counts; cols 128:256 = bank B Sign-inverted counts, host fixes)
    yp:       DRAM [n_rows, 128] f32 input AP
    yt:       DRAM [128, n_rows//128] f32 input AP (laid out on host so that
              column c*rpp+r on partition p holds y_true[c*128*rpp + p*rpp + r])
    iota_ap:  DRAM [128, 128] bf16, each partition = 0..127
    """
    nc = tc.nc
    chunk_rows = 128 * rpp
    n_chunks = n_rows // chunk_rows
    assert n_rows % chunk_rows == 0
    fd = rpp * C  # free dim of an x tile

    # chunk view: [n_chunks, 128p, rpp*C]
    yp_v = yp.rearrange("(c p r) k -> c p (r k)", p=128, r=rpp)

    const_pool = ctx.enter_context(tc.tile_pool(name="const", bufs=1))
    x_pool = ctx.enter_context(tc.tile_pool(name="x", bufs=bufs_x))
    m_pool = ctx.enter_context(tc.tile_pool(name="m", bufs=2 * bufs_st))
    s_pool = ctx.enter_context(tc.tile_pool(name="s", bufs=bufs_st))
    t_pool = ctx.enter_context(tc.tile_pool(name="t", bufs=bufs_st))
    psum_pool = ctx.enter_context(tc.tile_pool(name="psum", bufs=1, space="PSUM"))
    out_pool = ctx.enter_context(tc.tile_pool(name="out", bufs=1))

    iota_sb = const_pool.tile([128, C], BF16, tag="iota")
    nc.sync.dma_start(iota_sb[:], iota_ap)
    yt_sb = const_pool.tile([128, n_rows // 128], yt_dtype, tag="yt")
    nc.sync.dma_start(yt_sb[:], yt)
    trep_pool = ctx.enter_context(tc.tile_pool(name="trep", bufs=3))
    ones_sb = const_pool.tile([128, 16], BF16, tag="ones")
    nc.vector.memset(ones_sb[:], 1.0)

    conf_psum = psum_pool.tile([128, C], F32)
    confB_psum = psum_pool.tile([128, C], F32, tag="psumB")
    red_pool = ctx.enter_context(tc.tile_pool(name="red", bufs=2))

    # pemxr: PE replicates the per-row max into a step-1 SBUF operand so the
    # DVE mask runs as ONE tensor_tensor per chunk (group-broadcast 4D view)
    # instead of 16 drain-separated tensor_scalars.
    if dve_mask == "pemxr":
        ident_sb = const_pool.tile([128, C], BF16, tag="ident")
        nc.sync.dma_start(ident_sb[:], ident_ap)
        sel_sb = const_pool.tile([rpp, rpp * 32], BF16, tag="sel")
        nc.sync.dma_start(sel_sb[:], sel_ap)
        mxps_pool = ctx.enter_context(
            tc.tile_pool(name="mxps", bufs=2, space="PSUM"))
        mxr_pool = ctx.enter_context(tc.tile_pool(name="mxr", bufs=3))

    # chunk -> engine assignment for the mask stage ("mix" mode):
    # ACT handles act_frac of chunks via Sign (inverted mask, bank B + host
    # fix); GPSIMD handles gps_frac via whole-chunk tensor_tensor is_equal
    act_chunk = [False] * n_chunks
    gps_chunk = [False] * n_chunks
    if mask_mode == "mix" and act_pairs:
        for c in range(n_chunks):
            act_chunk[c] = (c // 2) % 2 == 0
    elif mask_mode == "mix":
        acc = gcc = 0.0
        for c in range(n_chunks):
            acc += act_frac
            if acc >= 1.0:
                acc -= 1.0
                act_chunk[c] = True
                continue
            gcc += gps_frac
            if gcc >= 1.0:
                gcc -= 1.0
                gps_chunk[c] = True
    a_list = [c for c in range(n_chunks) if not act_chunk[c]]
    b_list = [c for c in range(n_chunks) if act_chunk[c]]
    if mask_mode == "rsplit":
        a_list = list(range(n_chunks)) if act_slices < rpp else []
        b_list = list(range(n_chunks)) if act_slices > 0 else []

    def body():
        for c in range(n_chunks):
            x = x_pool.tile([128, fd], xdt, tag="x")
            if "dma" in stages:
                if dma_alt:
                    # two HWDGE rings: each engine loads the chunks whose
                    # mask it does NOT compute (sync ring for ACT chunks)
                    eng = nc.sync if (act_chunk[c] or c % 2 == 0) else nc.scalar
                    if mask_mode == "mix":
                        eng = nc.sync if act_chunk[c] else nc.scalar
                    eng.dma_start(x[:], yp_v[c])
                elif dma_split == 1:
                    nc.sync.dma_start(x[:], yp_v[c])
                else:
                    h = fd // dma_split
                    engs = [nc.sync, nc.tensor, nc.scalar, nc.vector]
                    for k in range(dma_split):
                        engs[k % len(engs)].dma_start(
                            x[:, k * h:(k + 1) * h], yp_v[c][:, k * h:(k + 1) * h])

            x3 = x[:].rearrange("p (r k) -> p r k", k=C)
            pemxr_c = (dve_mask == "pemxr" and mask_mode == "mix"
                       and not act_chunk[c] and "mask" in stages)
            mx = m_pool.tile([128, rpp], BF16 if pemxr_c else F32, tag="mx")
            if "reduce" in stages:
                if reduce_mode == "cascade4":
                    # 3 pairwise-max TT levels (2x mode) + small 1x reduce
                    src = x3
                    w = C // 2
                    for _ in range(3):
                        dst_t = red_pool.tile([128, rpp * w], xdt,
                                              tag=f"c4_{w}")
                        dst = dst_t[:].rearrange("p (r k) -> p r k", k=w)
                        nc.vector.tensor_tensor(
                            dst, src[:, :, 0:w], src[:, :, w:2 * w],
                            mybir.AluOpType.max)
                        src = dst
                        w //= 2
                    nc.vector.tensor_reduce(mx[:], src,
                                            axis=mybir.AxisListType.X,
                                            op=mybir.AluOpType.max)
                elif reduce_mode == "cascade7":
                    # all-TT pairwise-max tree at 2x; final level writes
                    # the f32 mx directly (last TT out dtype f32)
                    src = x3
                    w = C // 2
                    while w >= 1:
                        dst_t = red_pool.tile([128, rpp * w], xdt,
                                              tag=f"c7_{w}") if w > 1 else None
                        if w > 1:
                            dst = dst_t[:].rearrange("p (r k) -> p r k", k=w)
                        else:
                            dst = mx[:].unsqueeze(2)
                        nc.vector.tensor_tensor(
                            dst, src[:, :, 0:w], src[:, :, w:2 * w],
                            mybir.AluOpType.max)
                        src = dst
                        w //= 2
                elif reduce_mode == "cascade":
                    # pairwise-max TTs run in 2x DVE mode (bf16, step-1),
                    # shrinking the 1x tensor_reduce to a quarter of the data
                    h1, h2 = C // 2, C // 4
                    m1 = red_pool.tile([128, rpp * h1], xdt, tag="m1")
                    m1_3 = m1[:].rearrange("p (r k) -> p r k", k=h1)
                    nc.vector.tensor_tensor(
                        m1_3, x3[:, :, 0:h1], x3[:, :, h1:C],
                        mybir.AluOpType.max)
                    m2 = red_pool.tile([128, rpp * h2], xdt, tag="m2")
                    m2_3 = m2[:].rearrange("p (r k) -> p r k", k=h2)
                    nc.vector.tensor_tensor(
                        m2_3, m1_3[:, :, 0:h2], m1_3[:, :, h2:h1],
                        mybir.AluOpType.max)
                    nc.vector.tensor_reduce(mx[:], m2_3,
                                            axis=mybir.AxisListType.X,
                                            op=mybir.AluOpType.max)
                else:
                    nc.vector.tensor_reduce(mx[:], x3, axis=mybir.AxisListType.X,
                                            op=mybir.AluOpType.max)

            s_t = s_pool.tile([128, fd], BF16, tag="s")
            t_t = t_pool.tile([128, fd], BF16, tag="t")
            if "mask" in stages:
                if mask_mode == "rsplit":
                    k = act_slices
                    for r in range(k):
                        sl = slice(r * C, (r + 1) * C)
                        nc.scalar.activation(
                            s_t[:, sl], x[:, sl],
                            mybir.ActivationFunctionType.Sign,
                            bias=mx[:, r:r + 1], scale=-1.0)
                    for r in range(k, rpp):
                        sl = slice(r * C, (r + 1) * C)
                        nc.vector.tensor_scalar(
                            s_t[:, sl], x[:, sl], mx[:, r:r + 1], None,
                            mybir.AluOpType.is_equal)
                elif mask_mode == "mix" and act_chunk[c]:
                    # inverted mask on ACT: Sign(max - x) = 0 at argmax, 1 else
                    for r in range(rpp):
                        sl = slice(r * C, (r + 1) * C)
                        nc.scalar.activation(
                            s_t[:, sl], x[:, sl],
                            mybir.ActivationFunctionType.Sign,
                            bias=mx[:, r:r + 1], scale=-1.0)
                elif pemxr_c:
                    # PE: mxT = mx^T [rpp, 128] (psum); ACT: copy to SBUF;
                    # PE: mxr = mxT^T-select -> [128, rpp*32] f32 psum
                    # (mxr[p, r*32+j] = mx[p, r]); ACT: copy to SBUF bf16;
                    # DVE: one 4D TT is_equal vs group-broadcast mxr.
                    mxT_sb = mxr_pool.tile([rpp, 128], BF16, tag="mxT_sb")
                    if mxt_dma:
                        # strided DMA does the tiny transpose on the idle
                        # SP ring; frees the ACT psum->sbuf copy and the
                        # PE transpose (xbar path needs 128-divisible src)
                        nc.sync.dma_start(
                            mxT_sb[:], mx[:].rearrange("a b -> b a"))
                    else:
                        mxT_ps = mxps_pool.tile([rpp, 128], BF16, tag="mxT")
                        nc.tensor.transpose(mxT_ps[:], mx[:], ident_sb[:])
                        nc.scalar.copy(mxT_sb[:], mxT_ps[:])
                    mxr_sb = mxr_pool.tile([128, rpp * 32], BF16, tag="mxr_sb")
                    # one matmul per 512-elem PSUM bank (ISA free-dim limit)
                    n_mm = (rpp * 32 + 511) // 512
                    w_mm = rpp * 32 // n_mm
                    for i in range(n_mm):
                        sl_m = slice(i * w_mm, (i + 1) * w_mm)
                        mxr_ps = mxps_pool.tile([128, w_mm], F32,
                                                tag=f"mxr{i}")
                        nc.tensor.matmul(mxr_ps[:], mxT_sb[:],
                                         sel_sb[:, sl_m],
                                         start=True, stop=True)
                        nc.scalar.copy(mxr_sb[:, sl_m], mxr_ps[:])
                    x4 = x[:].rearrange("p (r g k) -> p r g k", g=4, k=32)
                    s4 = s_t[:].rearrange("p (r g k) -> p r g k", g=4, k=32)
                    mxr4 = (mxr_sb[:].rearrange("p (r k) -> p r k", k=32)
                            .unsqueeze(2).broadcast_to([128, rpp, 4, 32]))
                    nc.vector.tensor_tensor(s4, x4, mxr4,
                                            mybir.AluOpType.is_equal)
                elif mask_mode in ("ttb", "mix") and dve_mask == "ts16":
                    # per-row tensor_scalar: single-src op, hits DVE 4x mode
                    # at bf16 (the broadcast TT would fall to 1x: stride-0
                    # innermost on the mx operand disables 2x_1p)
                    for r in range(rpp):
                        sl = slice(r * C, (r + 1) * C)
                        nc.vector.tensor_scalar(
                            s_t[:, sl], x[:, sl], mx[:, r:r + 1], None,
                            mybir.AluOpType.is_equal)
                elif mask_mode in ("ttb", "mix"):
                    mx_b = mx[:].unsqueeze(2).broadcast_to([128, rpp, C])
                    eng = nc.gpsimd if gps_chunk[c] else nc.vector
                    eng.tensor_tensor(
                        s_t[:].rearrange("p (r k) -> p r k", k=C),
                        x3, mx_b, mybir.AluOpType.is_equal)
                else:
                    eng = nc.gpsimd if mask_mode == "gps" else nc.vector
                    for r in range(rpp):
                        sl = slice(r * C, (r + 1) * C)
                        eng.tensor_scalar(
                            s_t[:, sl], x[:, sl], mx[:, r:r + 1], None,
                            mybir.AluOpType.is_equal)
            if "onehot" in stages:
                if onehot_mode == "ts":
                    for r in range(rpp):
                        sl = slice(r * C, (r + 1) * C)
                        nc.vector.tensor_scalar(
                            t_t[:, sl], iota_sb[:],
                            yt_sb[:, c * rpp + r:c * rpp + r + 1],
                            None, mybir.AluOpType.is_equal)
                elif onehot_mode == "scatter":
                    # gpsimd local_scatter: per-partition one-hot build.
                    # yt holds host-precomputed int16 idx = (r%8)*128 + t.
                    half = 1024  # num_elems per call (must be < 2048)
                    rows_per_half = half // C  # 8
                    n_half = fd // half
                    for h in range(n_half):
                        nc.gpsimd.local_scatter(
                            t_t[:, h * half:(h + 1) * half],
                            ones_sb[:, :rows_per_half],
                            yt_sb[:, c * rpp + h * rows_per_half:
                                  c * rpp + (h + 1) * rows_per_half],
                            channels=128, num_elems=half,
                            num_idxs=rows_per_half)
                elif onehot_mode == "trep_tt":
                    # ACT materializes t replicated along the class dim;
                    # DVE compares against iota at bf16 2x
                    t_rep = trep_pool.tile([128, fd], BF16, tag="trep")
                    yt_bcast = (yt_sb[:, c * rpp:(c + 1) * rpp]
                                .unsqueeze(2).broadcast_to([128, rpp, C]))
                    nc.scalar.copy(t_rep[:].rearrange("p (r k) -> p r k", k=C),
                                   yt_bcast)
                    iota_b = (iota_sb[:].unsqueeze(1)
                              .broadcast_to([128, rpp, C]))
                    nc.vector.tensor_tensor(
                        t_t[:].rearrange("p (r k) -> p r k", k=C),
                        t_rep[:].rearrange("p (r k) -> p r k", k=C),
                        iota_b, mybir.AluOpType.is_equal)
                else:
                    raise ValueError(onehot_mode)
            if "matmul" in stages:
                if mask_mode == "rsplit":
                    k = act_slices
                    for r in range(rpp):
                        sl = slice(r * C, (r + 1) * C)
                        if r < k:
                            nc.tensor.matmul(
                                confB_psum[:], t_t[:, sl], s_t[:, sl],
                                start=(c == 0 and r == 0),
                                stop=(c == n_chunks - 1 and r == k - 1))
                        else:
                            nc.tensor.matmul(
                                conf_psum[:], t_t[:, sl], s_t[:, sl],
                                start=(c == 0 and r == k),
                                stop=(c == n_chunks - 1 and r == rpp - 1))
                else:
                    if act_chunk[c]:
                        psum, first_c, last_c = confB_psum, b_list[0], b_list[-1]
                    else:
                        psum, first_c, last_c = conf_psum, a_list[0], a_list[-1]

                    def emit_mm(c=c, psum=psum, first_c=first_c, last_c=last_c,
                                t_t=t_t, s_t=s_t):
                        for r in range(rpp):
                            sl = slice(r * C, (r + 1) * C)
                            nc.tensor.matmul(
                                psum[:], t_t[:, sl], s_t[:, sl],
                                start=(c == first_c and r == 0),
                                stop=(c == last_c and r == rpp - 1))

                    if dve_mask == "pemxr":
                        # delay this chunk's conf matmuls by one chunk so the
                        # in-order PE stream runs chunk c+1's transpose/select
                        # matmuls while chunk c's mask is still being built
                        pending_mm.append(emit_mm)
                        if len(pending_mm) > 1:
                            pending_mm.pop(0)()
                    else:
                        emit_mm()
        while pending_mm:
            pending_mm.pop(0)()

    pending_mm = []
    if reps == 1:
        body()
    else:
        hints = (tuple(mybir.EngineType[e] for e in
                       ("DVE", "Activation", "PE", "SP", "Pool"))
                 if loop_hints else ())
        # unroll>1 amortizes any pipeline drain at the hardware-loop
        # boundary; total logical iterations stay exactly `reps`
        with tc.For_i(0, reps // unroll, 1, hint_engines=hints):
            for _ in range(unroll):
                body()
        for _ in range(reps % unroll):
            body()

    conf_sb = out_pool.tile([128, 2 * C], F32)
    if "matmul" in stages and a_list:
        nc.scalar.copy(conf_sb[:, :C], conf_psum[:])
    else:
        nc.vector.memset(conf_sb[:, :C], 0.0)
    if "matmul" in stages and b_list:
        nc.scalar.copy(conf_sb[:, C:], confB_psum[:])
    else:
        nc.vector.memset(conf_sb[:, C:], 0.0)
    nc.sync.dma_start(conf_out, conf_sb[:])


def _host_layout_ytrue(yt_shard, rpp=8, np_dtype=np.float32):
    """[SHARD] ints -> [128, SHARD//128] in the kernel's expected layout."""
    n_chunks = yt_shard.shape[0] // (128 * rpp)
    return (yt_shard.reshape(n_chunks, 128, rpp)
            .transpose(1, 0, 2)
            .reshape(128, -1)
            .astype(np_dtype))


def _host_layout_scatter_idx(yt_shard, rpp=8):
    """[SHARD] ints -> int16 [128, SHARD//128]: value (r%8)*128 + t in the
    kernel's (p, c*rpp+r) layout, for gpsimd local_scatter one-hot builds."""
    lay = _host_layout_ytrue(yt_shard, rpp, np.int64)
    ncols = lay.shape[1]
    offs = ((np.arange(ncols) % rpp) % 8) * C
    return (lay + offs[None, :]).astype(np.int16)


def _iota_np():
    return np.tile(np.arange(C, dtype=ml_dtypes.bfloat16), (128, 1))


_compiled = {}

# Best measured config on trn2 (see ablate.py): ~180us/iter vs ~103us bf16
# DMA floor per core (DVE+ACT compute-balanced).
BEST = dict(rpp=32, yt_dtype=mybir.dt.int16, onehot_mode="scatter",
            mask_mode="mix", act_frac=0.40, bufs_x=6, bufs_st=8,
            xdt=BF16, reduce_mode="cascade", dve_mask="pemxr", unroll=3)


def _get_program(rpp=8, reps=1,
                 stages=("dma", "reduce", "mask", "onehot", "matmul"),
                 yt_dtype=F32, onehot_mode="ts", mask_mode="dve", dma_split=1,
                 act_frac=0.5, gps_frac=0.0, bufs_x=3, bufs_st=3,
                 act_slices=8, loop_hints=False, dma_alt=False, act_pairs=False,
                 xdt=F32, reduce_mode="direct", dve_mask="tt", unroll=1,
                 mxt_dma=False):
    if reps == 1:
        unroll = 1  # unroll only affects the For_i repeat loop
    key = (rpp, reps, tuple(stages), yt_dtype, onehot_mode, mask_mode, dma_split,
           act_frac, gps_frac, bufs_x, bufs_st, act_slices, loop_hints, dma_alt,
           xdt, reduce_mode, dve_mask, unroll, mxt_dma)
    if key in _compiled:
        return _compiled[key]
    nc = bacc.Bacc("TRN2", target_bir_lowering=False, debug=False)
    yp = nc.dram_tensor("yp", [SHARD, C], xdt, kind="ExternalInput").ap()
    yt = nc.dram_tensor("yt", [128, SHARD // 128], yt_dtype,
                        kind="ExternalInput").ap()
    iota_d = nc.dram_tensor("iota", [128, C], BF16, kind="ExternalInput").ap()
    ident_d = sel_d = None
    if dve_mask == "pemxr":
        ident_d = nc.dram_tensor("ident", [128, C], BF16,
                                 kind="ExternalInput").ap()
        sel_d = nc.dram_tensor("sel", [rpp, rpp * 32], BF16,
                               kind="ExternalInput").ap()
    conf = nc.dram_tensor("conf", [128, 2 * C], F32, kind="ExternalOutput").ap()
    with tile.TileContext(nc) as tc:
        with ExitStack() as ctx:
            build_conf_kernel(ctx, tc, conf, yp, yt, iota_d, SHARD, rpp=rpp,
                              reps=reps, stages=stages, yt_dtype=yt_dtype,
                              onehot_mode=onehot_mode, mask_mode=mask_mode,
                              dma_split=dma_split, act_frac=act_frac,
                              gps_frac=gps_frac, bufs_x=bufs_x, bufs_st=bufs_st,
                              act_slices=act_slices, loop_hints=loop_hints,
                              dma_alt=dma_alt, act_pairs=act_pairs,
                              xdt=xdt, reduce_mode=reduce_mode, dve_mask=dve_mask,
                              ident_ap=ident_d, sel_ap=sel_d, unroll=unroll,
                              mxt_dma=mxt_dma)
    nc.compile()
    _compiled[key] = nc
    return nc


def conf_from_banks(res256):
    """[128,256] per-core result -> [128,128] f64 confusion counts."""
    res256 = res256.astype(np.float64)
    conf_a = res256[:, :C]
    m_b = res256[:, C:]
    cnt_b = m_b.sum(axis=1) / (C - 1)
    conf_b = cnt_b[:, None] - m_b
    return conf_a + conf_b


def f1_from_conf(conf_f):
    """Replicates the reference's (quirky) F1 reduction on a [128,128] f32
    confusion matrix."""
    conf_f = conf_f.astype(np.float32)
    TP = np.diagonal(conf_f).astype(np.float32)
    FP = np.float32(C - 1) * conf_f[:, 1] + conf_f[:, 0]
    FN = np.float32(C - 1) * conf_f[1, :] + conf_f[0, :]
    sensitivity = TP / (TP + FN + EPS)
    precision = TP / (TP + FP + EPS)
    f1 = np.float32(2.0) * (precision * sensitivity / (precision + sensitivity + EPS))
    return np.array(np.mean(f1), dtype=np.float32)


def make_in_maps(y_pred, y_true, cfg=None):
    """Shard + lay out the full inputs for the 8-core SPMD program."""
    if cfg is None:
        cfg = BEST
    iota_np = _iota_np()
    if cfg.get("xdt", F32) == BF16:
        # stream the scores at bf16: halves HBM traffic; argmax ties from
        # the rounding shift the conf counts by ~0.4% of rows (measured F1
        # rel err 1.0e-3, tolerance 2e-2)
        y_pred = y_pred.astype(ml_dtypes.bfloat16)
    yp_sh = y_pred.reshape(N_CORES, SHARD, C)
    yt_sh = y_true.reshape(N_CORES, SHARD)
    rpp = cfg["rpp"]
    extra = {}
    if cfg.get("dve_mask") == "pemxr":
        extra["ident"] = np.eye(C, dtype=ml_dtypes.bfloat16)
        sel = np.zeros((rpp, rpp * 32), dtype=ml_dtypes.bfloat16)
        for q in range(rpp):
            sel[q, q * 32:(q + 1) * 32] = 1.0
        extra["sel"] = sel
    return [{
        "yp": yp_sh[i],
        "yt": _host_layout_scatter_idx(yt_sh[i], rpp),
        "iota": iota_np,
        **extra,
    } for i in range(N_CORES)]


def kernel(y_pred, y_true, _spmd_runner=None, **_ignored):
    y_pred = np.ascontiguousarray(np.asarray(y_pred), dtype=np.float32)
    y_true = np.asarray(y_true)
    assert y_pred.shape == (N_TOTAL, C)

    nc = _get_program(**BEST)
    in_maps = make_in_maps(y_pred, y_true)
    runner = _spmd_runner or bass_utils.run_bass_kernel_spmd
    res = runner(nc, in_maps, core_ids=list(range(N_CORES)))
    results = res.results if hasattr(res, "results") else res
    conf = np.zeros((128, C), dtype=np.float64)
    for r in results:
        conf += conf_from_banks(r["conf"])
    return f1_from_conf(conf.astype(np.float32))



# revision 37
# speedup vs baseline: 1.0076x; 1.0076x over previous
"""Trainium2 Bass kernel for nn_F1_67379446940315 (histogram_binning F1 metric).

Computes: pred = argmax(y_pred, axis=1); conf = scatter-add confusion matrix;
then the (quirky, faithful-to-reference) per-class F1 reduction to a scalar.

Strategy (8 NeuronCores, data-parallel over N; ~180us/iter vs ~103us bf16 DMA
floor per core at ~330 GB/s):
  - host converts y_pred to bf16 (halves HBM traffic; bf16 argmax rounding
    ties shift the conf counts by ~0.4% of rows -> F1 rel err ~1.7e-3,
    tolerance 2e-2)
  - each core streams its shard [131072, 128] bf16 in 1 MiB chunks
    [128 partitions, 32 rows x 128 classes]
  - per-row max via a 3-op DVE cascade: two pairwise-max tensor_tensors
    (2x bf16 mode) + one small 1x tensor_reduce -- ~35% cheaper than a
    single full-width 1x reduce
  - pred one-hot mask, split across engines (all are near-saturated):
    - 60% of chunks on DVE via ONE 4D tensor_tensor is_equal against a
      PE-materialized replicated row-max ("pemxr"): PE transposes mx,
      a selector matmul broadcasts it to [128, rpp*32] PSUM, ACT copies
      it to SBUF bf16; the group-broadcast TT then runs without the
      16 drain-separated per-row tensor_scalars it replaces
    - 40% on ACT (ScalarE): Sign(rowmax - x) per row slice = an INVERTED
      {0,1} mask into a second PSUM bank; the host undoes the inversion
      exactly (integer algebra)
  - true one-hot T built by GPSIMD local_scatter from host-precomputed
    int16 indices (r%8)*128 + y_true -- zero DVE/ACT cost
  - PE matmul accumulation: conf_psum[bank] += T_r^T @ S_r (contraction
    over the 128 rows on partitions), 32 matmuls per chunk, emitted with a
    one-chunk delay so the in-order PE stream interleaves the next chunk's
    transpose/selector matmuls instead of stalling on the mask
  - per-core [128, 256] (bank A | bank B) f32 DMA'd out; host reconstructs
    conf = A + (cntB - B), sums the 8 partials, and does the tiny F1
    reduction (negligible work, replicated per the sharding hint).

Engine budget per 1 MiB chunk (~5.6us at 180us/iter, DMA floor 3.2us):
DVE cascade 2.85 + mask 0.6*4.4 = 5.5; ACT 0.4*12.7 + copies 0.77 = 5.9;
PE 32 matmuls + transpose/select ~4; Pool 2 local_scatters ~0.5.
"""

import numpy as np
import ml_dtypes
from contextlib import ExitStack

import concourse.bass as bass
import concourse.bacc as bacc
import concourse.tile as tile
from concourse import mybir
from concourse import bass_utils

N_TOTAL = 1048576
C = 128
N_CORES = 8
SHARD = N_TOTAL // N_CORES  # 131072
EPS = np.float32(1e-12)

BF16 = mybir.dt.bfloat16
F32 = mybir.dt.float32


def build_conf_kernel(ctx, tc, conf_out, yp, yt, iota_ap, n_rows, rpp=8, reps=1,
                      stages=("dma", "reduce", "mask", "onehot", "matmul"),
                      yt_dtype=F32, onehot_mode="ts", mask_mode="dve",
                      dma_split=1, act_frac=0.5, gps_frac=0.0, bufs_x=3, bufs_st=3,
                      act_slices=8, loop_hints=False, dma_alt=False, act_pairs=False,
                      xdt=F32, reduce_mode="direct", dve_mask="tt",
                      ident_ap=None, sel_ap=None, unroll=1, mxt_dma=False,
                      psum_mask=False):
    """Emit the per-core confusion-matrix kernel.

    conf_out: DRAM [128,256] f32 output AP (cols 0:128 = bank A is_equal
              counts; cols 128:256 = bank B Sign-inverted counts, host fixes)
    yp:       DRAM [n_rows, 128] f32 input AP
    yt:       DRAM [128, n_rows//128] f32 input AP (laid out on host so that
              column c*rpp+r on partition p holds y_true[c*128*rpp + p*rpp + r])
    iota_ap:  DRAM [128, 128] bf16, each partition = 0..127
    """
    nc = tc.nc
    chunk_rows = 128 * rpp
    n_chunks = n_rows // chunk_rows
    assert n_rows % chunk_rows == 0
    fd = rpp * C  # free dim of an x tile

    # chunk view: [n_chunks, 128p, rpp*C]
    yp_v = yp.rearrange("(c p r) k -> c p (r k)", p=128, r=rpp)

    const_pool = ctx.enter_context(tc.tile_pool(name="const", bufs=1))
    x_pool = ctx.enter_context(tc.tile_pool(name="x", bufs=bufs_x))
    m_pool = ctx.enter_context(tc.tile_pool(name="m", bufs=2 * bufs_st))
    s_pool = ctx.enter_context(tc.tile_pool(name="s", bufs=bufs_st))
    t_pool = ctx.enter_context(tc.tile_pool(name="t", bufs=bufs_st))
    psum_pool = ctx.enter_context(tc.tile_pool(name="psum", bufs=1, space="PSUM"))
    out_pool = ctx.enter_context(tc.tile_pool(name="out", bufs=1))

    iota_sb = const_pool.tile([128, C], BF16, tag="iota")
    nc.sync.dma_start(iota_sb[:], iota_ap)
    yt_sb = const_pool.tile([128, n_rows // 128], yt_dtype, tag="yt")
    nc.sync.dma_start(yt_sb[:], yt)
    trep_pool = ctx.enter_context(tc.tile_pool(name="trep", bufs=3))
    ones_sb = const_pool.tile([128, 16], BF16, tag="ones")
    nc.vector.memset(ones_sb[:], 1.0)

    conf_psum = psum_pool.tile([128, C], F32)
    confB_psum = psum_pool.tile([128, C], F32, tag="psumB")
    red_pool = ctx.enter_context(tc.tile_pool(name="red", bufs=2))

    # pemxr: PE replicates the per-row max into a step-1 SBUF operand so the
    # DVE mask runs as ONE tensor_tensor per chunk (group-broadcast 4D view)
    # instead of 16 drain-separated tensor_scalars.
    if dve_mask == "pemxr":
        ident_sb = const_pool.tile([128, C], BF16, tag="ident")
        nc.sync.dma_start(ident_sb[:], ident_ap)
        sel_sb = const_pool.tile([rpp, rpp * 32], BF16, tag="sel")
        nc.sync.dma_start(sel_sb[:], sel_ap)
        mxps_pool = ctx.enter_context(
            tc.tile_pool(name="mxps", bufs=2, space="PSUM"))
        mxr_pool = ctx.enter_context(tc.tile_pool(name="mxr", bufs=3))

    # chunk -> engine assignment for the mask stage ("mix" mode):
    # ACT handles act_frac of chunks via Sign (inverted mask, bank B + host
    # fix); GPSIMD handles gps_frac via whole-chunk tensor_tensor is_equal
    act_chunk = [False] * n_chunks
    gps_chunk = [False] * n_chunks
    if mask_mode == "mix" and act_pairs:
        for c in range(n_chunks):
            act_chunk[c] = (c // 2) % 2 == 0
    elif mask_mode == "mix":
        acc = gcc = 0.0
        for c in range(n_chunks):
            acc += act_frac
            if acc >= 1.0:
                acc -= 1.0
                act_chunk[c] = True
                continue
            gcc += gps_frac
            if gcc >= 1.0:
                gcc -= 1.0
                gps_chunk[c] = True
    a_list = [c for c in range(n_chunks) if not act_chunk[c]]
    b_list = [c for c in range(n_chunks) if act_chunk[c]]
    if mask_mode == "rsplit":
        a_list = list(range(n_chunks)) if act_slices < rpp else []
        b_list = list(range(n_chunks)) if act_slices > 0 else []

    def body():
        for c in range(n_chunks):
            x = x_pool.tile([128, fd], xdt, tag="x")
            if "dma" in stages:
                if dma_alt:
                    # two HWDGE rings: each engine loads the chunks whose
                    # mask it does NOT compute (sync ring for ACT chunks)
                    eng = nc.sync if (act_chunk[c] or c % 2 == 0) else nc.scalar
                    if mask_mode == "mix":
                        eng = nc.sync if act_chunk[c] else nc.scalar
                    eng.dma_start(x[:], yp_v[c])
                elif dma_split == 1:
                    nc.sync.dma_start(x[:], yp_v[c])
                else:
                    h = fd // dma_split
                    engs = [nc.sync, nc.tensor, nc.scalar, nc.vector]
                    for k in range(dma_split):
                        engs[k % len(engs)].dma_start(
                            x[:, k * h:(k + 1) * h], yp_v[c][:, k * h:(k + 1) * h])

            x3 = x[:].rearrange("p (r k) -> p r k", k=C)
            pemxr_c = (dve_mask == "pemxr" and mask_mode == "mix"
                       and not act_chunk[c] and "mask" in stages)
            mx = m_pool.tile([128, rpp], BF16 if pemxr_c else F32, tag="mx")
            if "reduce" in stages:
                if reduce_mode == "cascade4":
                    # 3 pairwise-max TT levels (2x mode) + small 1x reduce
                    src = x3
                    w = C // 2
                    for _ in range(3):
                        dst_t = red_pool.tile([128, rpp * w], xdt,
                                              tag=f"c4_{w}")
                        dst = dst_t[:].rearrange("p (r k) -> p r k", k=w)
                        nc.vector.tensor_tensor(
                            dst, src[:, :, 0:w], src[:, :, w:2 * w],
                            mybir.AluOpType.max)
                        src = dst
                        w //= 2
                    nc.vector.tensor_reduce(mx[:], src,
                                            axis=mybir.AxisListType.X,
                                            op=mybir.AluOpType.max)
                elif reduce_mode == "cascade7":
                    # all-TT pairwise-max tree at 2x; final level writes
                    # the f32 mx directly (last TT out dtype f32)
                    src = x3
                    w = C // 2
                    while w >= 1:
                        dst_t = red_pool.tile([128, rpp * w], xdt,
                                              tag=f"c7_{w}") if w > 1 else None
                        if w > 1:
                            dst = dst_t[:].rearrange("p (r k) -> p r k", k=w)
                        else:
                            dst = mx[:].unsqueeze(2)
                        nc.vector.tensor_tensor(
                            dst, src[:, :, 0:w], src[:, :, w:2 * w],
                            mybir.AluOpType.max)
                        src = dst
                        w //= 2
                elif reduce_mode == "cascade":
                    # pairwise-max TTs run in 2x DVE mode (bf16, step-1),
                    # shrinking the 1x tensor_reduce to a quarter of the data
                    h1, h2 = C // 2, C // 4
                    m1 = red_pool.tile([128, rpp * h1], xdt, tag="m1")
                    m1_3 = m1[:].rearrange("p (r k) -> p r k", k=h1)
                    nc.vector.tensor_tensor(
                        m1_3, x3[:, :, 0:h1], x3[:, :, h1:C],
                        mybir.AluOpType.max)
                    m2 = red_pool.tile([128, rpp * h2], xdt, tag="m2")
                    m2_3 = m2[:].rearrange("p (r k) -> p r k", k=h2)
                    nc.vector.tensor_tensor(
                        m2_3, m1_3[:, :, 0:h2], m1_3[:, :, h2:h1],
                        mybir.AluOpType.max)
                    nc.vector.tensor_reduce(mx[:], m2_3,
                                            axis=mybir.AxisListType.X,
                                            op=mybir.AluOpType.max)
                else:
                    nc.vector.tensor_reduce(mx[:], x3, axis=mybir.AxisListType.X,
                                            op=mybir.AluOpType.max)

            s_t = s_pool.tile([128, fd], BF16, tag="s")
            t_t = t_pool.tile([128, fd], BF16, tag="t")
            if "mask" in stages:
                if mask_mode == "rsplit":
                    k = act_slices
                    for r in range(k):
                        sl = slice(r * C, (r + 1) * C)
                        nc.scalar.activation(
                            s_t[:, sl], x[:, sl],
                            mybir.ActivationFunctionType.Sign,
                            bias=mx[:, r:r + 1], scale=-1.0)
                    for r in range(k, rpp):
                        sl = slice(r * C, (r + 1) * C)
                        nc.vector.tensor_scalar(
                            s_t[:, sl], x[:, sl], mx[:, r:r + 1], None,
                            mybir.AluOpType.is_equal)
                elif mask_mode == "mix" and act_chunk[c]:
                    # inverted mask on ACT: Sign(max - x) = 0 at argmax, 1 else
                    for r in range(rpp):
                        sl = slice(r * C, (r + 1) * C)
                        nc.scalar.activation(
                            s_t[:, sl], x[:, sl],
                            mybir.ActivationFunctionType.Sign,
                            bias=mx[:, r:r + 1], scale=-1.0)
                elif pemxr_c:
                    # PE: mxT = mx^T [rpp, 128] (psum); ACT: copy to SBUF;
                    # PE: mxr = mxT^T-select -> [128, rpp*32] f32 psum
                    # (mxr[p, r*32+j] = mx[p, r]); ACT: copy to SBUF bf16;
                    # DVE: one 4D TT is_equal vs group-broadcast mxr.
                    mxT_sb = mxr_pool.tile([rpp, 128], BF16, tag="mxT_sb")
                    if mxt_dma:
                        # strided DMA does the tiny transpose on the idle
                        # SP ring; frees the ACT psum->sbuf copy and the
                        # PE transpose (xbar path needs 128-divisible src)
                        nc.sync.dma_start(
                            mxT_sb[:], mx[:].rearrange("a b -> b a"))
                    else:
                        mxT_ps = mxps_pool.tile([rpp, 128], BF16, tag="mxT")
                        nc.tensor.transpose(mxT_ps[:], mx[:], ident_sb[:])
                        nc.scalar.copy(mxT_sb[:], mxT_ps[:])
                    x4 = x[:].rearrange("p (r g k) -> p r g k", g=4, k=32)
                    s4 = s_t[:].rearrange("p (r g k) -> p r g k", g=4, k=32)
                    # one matmul per 512-elem PSUM bank (ISA free-dim limit)
                    n_mm = (rpp * 32 + 511) // 512
                    w_mm = rpp * 32 // n_mm
                    if psum_mask:
                        # mask TT is 1x regardless, and PSUM operands stay
                        # 1x: read mxr straight from bf16 PSUM, skipping
                        # both ACT psum->sbuf copies
                        rows_mm = w_mm // 32
                        for i in range(n_mm):
                            sl_m = slice(i * w_mm, (i + 1) * w_mm)
                            mxr_ps = mxps_pool.tile([128, w_mm], F32,
                                                    tag=f"mxrb{i}")
                            nc.tensor.matmul(mxr_ps[:], mxT_sb[:],
                                             sel_sb[:, sl_m],
                                             start=True, stop=True)
                            r0 = i * rows_mm
                            mxr4 = (mxr_ps[:]
                                    .rearrange("p (r k) -> p r k", k=32)
                                    .unsqueeze(2)
                                    .broadcast_to([128, rows_mm, 4, 32]))
                            nc.vector.tensor_tensor(
                                s4[:, r0:r0 + rows_mm], x4[:, r0:r0 + rows_mm],
                                mxr4, mybir.AluOpType.is_equal)
                    else:
                        mxr_sb = mxr_pool.tile([128, rpp * 32], BF16,
                                               tag="mxr_sb")
                        for i in range(n_mm):
                            sl_m = slice(i * w_mm, (i + 1) * w_mm)
                            mxr_ps = mxps_pool.tile([128, w_mm], F32,
                                                    tag=f"mxr{i}")
                            nc.tensor.matmul(mxr_ps[:], mxT_sb[:],
                                             sel_sb[:, sl_m],
                                             start=True, stop=True)
                            nc.scalar.copy(mxr_sb[:, sl_m], mxr_ps[:])
                        mxr4 = (mxr_sb[:].rearrange("p (r k) -> p r k", k=32)
                                .unsqueeze(2).broadcast_to([128, rpp, 4, 32]))
                        nc.vector.tensor_tensor(s4, x4, mxr4,
                                                mybir.AluOpType.is_equal)
                elif mask_mode in ("ttb", "mix") and dve_mask == "ts16":
                    # per-row tensor_scalar: single-src op, hits DVE 4x mode
                    # at bf16 (the broadcast TT would fall to 1x: stride-0
                    # innermost on the mx operand disables 2x_1p)
                    for r in range(rpp):
                        sl = slice(r * C, (r + 1) * C)
                        nc.vector.tensor_scalar(
                            s_t[:, sl], x[:, sl], mx[:, r:r + 1], None,
                            mybir.AluOpType.is_equal)
                elif mask_mode in ("ttb", "mix"):
                    mx_b = mx[:].unsqueeze(2).broadcast_to([128, rpp, C])
                    eng = nc.gpsimd if gps_chunk[c] else nc.vector
                    eng.tensor_tensor(
                        s_t[:].rearrange("p (r k) -> p r k", k=C),
                        x3, mx_b, mybir.AluOpType.is_equal)
                else:
                    eng = nc.gpsimd if mask_mode == "gps" else nc.vector
                    for r in range(rpp):
                        sl = slice(r * C, (r + 1) * C)
                        eng.tensor_scalar(
                            s_t[:, sl], x[:, sl], mx[:, r:r + 1], None,
                            mybir.AluOpType.is_equal)
            if "onehot" in stages:
                if onehot_mode == "ts":
                    for r in range(rpp):
                        sl = slice(r * C, (r + 1) * C)
                        nc.vector.tensor_scalar(
                            t_t[:, sl], iota_sb[:],
                            yt_sb[:, c * rpp + r:c * rpp + r + 1],
                            None, mybir.AluOpType.is_equal)
                elif onehot_mode == "scatter":
                    # gpsimd local_scatter: per-partition one-hot build.
                    # yt holds host-precomputed int16 idx = (r%8)*128 + t.
                    half = 1024  # num_elems per call (must be < 2048)
                    rows_per_half = half // C  # 8
                    n_half = fd // half
                    for h in range(n_half):
                        nc.gpsimd.local_scatter(
                            t_t[:, h * half:(h + 1) * half],
                            ones_sb[:, :rows_per_half],
                            yt_sb[:, c * rpp + h * rows_per_half:
                                  c * rpp + (h + 1) * rows_per_half],
                            channels=128, num_elems=half,
                            num_idxs=rows_per_half)
                elif onehot_mode == "trep_tt":
                    # ACT materializes t replicated along the class dim;
                    # DVE compares against iota at bf16 2x
                    t_rep = trep_pool.tile([128, fd], BF16, tag="trep")
                    yt_bcast = (yt_sb[:, c * rpp:(c + 1) * rpp]
                                .unsqueeze(2).broadcast_to([128, rpp, C]))
                    nc.scalar.copy(t_rep[:].rearrange("p (r k) -> p r k", k=C),
                                   yt_bcast)
                    iota_b = (iota_sb[:].unsqueeze(1)
                              .broadcast_to([128, rpp, C]))
                    nc.vector.tensor_tensor(
                        t_t[:].rearrange("p (r k) -> p r k", k=C),
                        t_rep[:].rearrange("p (r k) -> p r k", k=C),
                        iota_b, mybir.AluOpType.is_equal)
                else:
                    raise ValueError(onehot_mode)
            if "matmul" in stages:
                if mask_mode == "rsplit":
                    k = act_slices
                    for r in range(rpp):
                        sl = slice(r * C, (r + 1) * C)
                        if r < k:
                            nc.tensor.matmul(
                                confB_psum[:], t_t[:, sl], s_t[:, sl],
                                start=(c == 0 and r == 0),
                                stop=(c == n_chunks - 1 and r == k - 1))
                        else:
                            nc.tensor.matmul(
                                conf_psum[:], t_t[:, sl], s_t[:, sl],
                                start=(c == 0 and r == k),
                                stop=(c == n_chunks - 1 and r == rpp - 1))
                else:
                    if act_chunk[c]:
                        psum, first_c, last_c = confB_psum, b_list[0], b_list[-1]
                    else:
                        psum, first_c, last_c = conf_psum, a_list[0], a_list[-1]

                    def emit_mm(c=c, psum=psum, first_c=first_c, last_c=last_c,
                                t_t=t_t, s_t=s_t):
                        for r in range(rpp):
                            sl = slice(r * C, (r + 1) * C)
                            nc.tensor.matmul(
                                psum[:], t_t[:, sl], s_t[:, sl],
                                start=(c == first_c and r == 0),
                                stop=(c == last_c and r == rpp - 1))

                    if dve_mask == "pemxr":
                        # delay this chunk's conf matmuls by one chunk so the
                        # in-order PE stream runs chunk c+1's transpose/select
                        # matmuls while chunk c's mask is still being built
                        pending_mm.append(emit_mm)
                        if len(pending_mm) > 1:
                            pending_mm.pop(0)()
                    else:
                        emit_mm()
        while pending_mm:
            pending_mm.pop(0)()

    pending_mm = []
    if reps == 1:
        body()
    else:
        hints = (tuple(mybir.EngineType[e] for e in
                       ("DVE", "Activation", "PE", "SP", "Pool"))
                 if loop_hints else ())
        # unroll>1 amortizes any pipeline drain at the hardware-loop
        # boundary; total logical iterations stay exactly `reps`
        with tc.For_i(0, reps // unroll, 1, hint_engines=hints):
            for _ in range(unroll):
                body()
        for _ in range(reps % unroll):
            body()

    conf_sb = out_pool.tile([128, 2 * C], F32)
    if "matmul" in stages and a_list:
        nc.scalar.copy(conf_sb[:, :C], conf_psum[:])
    else:
        nc.vector.memset(conf_sb[:, :C], 0.0)
    if "matmul" in stages and b_list:
        nc.scalar.copy(conf_sb[:, C:], confB_psum[:])
    else:
        nc.vector.memset(conf_sb[:, C:], 0.0)
    nc.sync.dma_start(conf_out, conf_sb[:])


def _host_layout_ytrue(yt_shard, rpp=8, np_dtype=np.float32):
    """[SHARD] ints -> [128, SHARD//128] in the kernel's expected layout."""
    n_chunks = yt_shard.shape[0] // (128 * rpp)
    return (yt_shard.reshape(n_chunks, 128, rpp)
            .transpose(1, 0, 2)
            .reshape(128, -1)
            .astype(np_dtype))


def _host_layout_scatter_idx(yt_shard, rpp=8):
    """[SHARD] ints -> int16 [128, SHARD//128]: value (r%8)*128 + t in the
    kernel's (p, c*rpp+r) layout, for gpsimd local_scatter one-hot builds."""
    lay = _host_layout_ytrue(yt_shard, rpp, np.int64)
    ncols = lay.shape[1]
    offs = ((np.arange(ncols) % rpp) % 8) * C
    return (lay + offs[None, :]).astype(np.int16)


def _iota_np():
    return np.tile(np.arange(C, dtype=ml_dtypes.bfloat16), (128, 1))


_compiled = {}

# Best measured config on trn2 (see ablate.py): ~180us/iter vs ~103us bf16
# DMA floor per core (DVE+ACT compute-balanced).
BEST = dict(rpp=32, yt_dtype=mybir.dt.int16, onehot_mode="scatter",
            mask_mode="mix", act_frac=0.40, bufs_x=6, bufs_st=8,
            xdt=BF16, reduce_mode="cascade", dve_mask="pemxr", unroll=3)


def _get_program(rpp=8, reps=1,
                 stages=("dma", "reduce", "mask", "onehot", "matmul"),
                 yt_dtype=F32, onehot_mode="ts", mask_mode="dve", dma_split=1,
                 act_frac=0.5, gps_frac=0.0, bufs_x=3, bufs_st=3,
                 act_slices=8, loop_hints=False, dma_alt=False, act_pairs=False,
                 xdt=F32, reduce_mode="direct", dve_mask="tt", unroll=1,
                 mxt_dma=False, psum_mask=False):
    if reps == 1:
        unroll = 1  # unroll only affects the For_i repeat loop
    key = (rpp, reps, tuple(stages), yt_dtype, onehot_mode, mask_mode, dma_split,
           act_frac, gps_frac, bufs_x, bufs_st, act_slices, loop_hints, dma_alt,
           xdt, reduce_mode, dve_mask, unroll, mxt_dma, psum_mask)
    if key in _compiled:
        return _compiled[key]
    nc = bacc.Bacc("TRN2", target_bir_lowering=False, debug=False)
    yp = nc.dram_tensor("yp", [SHARD, C], xdt, kind="ExternalInput").ap()
    yt = nc.dram_tensor("yt", [128, SHARD // 128], yt_dtype,
                        kind="ExternalInput").ap()
    iota_d = nc.dram_tensor("iota", [128, C], BF16, kind="ExternalInput").ap()
    ident_d = sel_d = None
    if dve_mask == "pemxr":
        ident_d = nc.dram_tensor("ident", [128, C], BF16,
                                 kind="ExternalInput").ap()
        sel_d = nc.dram_tensor("sel", [rpp, rpp * 32], BF16,
                               kind="ExternalInput").ap()
    conf = nc.dram_tensor("conf", [128, 2 * C], F32, kind="ExternalOutput").ap()
    with tile.TileContext(nc) as tc:
        with ExitStack() as ctx:
            build_conf_kernel(ctx, tc, conf, yp, yt, iota_d, SHARD, rpp=rpp,
                              reps=reps, stages=stages, yt_dtype=yt_dtype,
                              onehot_mode=onehot_mode, mask_mode=mask_mode,
                              dma_split=dma_split, act_frac=act_frac,
                              gps_frac=gps_frac, bufs_x=bufs_x, bufs_st=bufs_st,
                              act_slices=act_slices, loop_hints=loop_hints,
                              dma_alt=dma_alt, act_pairs=act_pairs,
                              xdt=xdt, reduce_mode=reduce_mode, dve_mask=dve_mask,
                              ident_ap=ident_d, sel_ap=sel_d, unroll=unroll,
                              mxt_dma=mxt_dma, psum_mask=psum_mask)
    nc.compile()
    _compiled[key] = nc
    return nc


def conf_from_banks(res256):
    """[128,256] per-core result -> [128,128] f64 confusion counts."""
    res256 = res256.astype(np.float64)
    conf_a = res256[:, :C]
    m_b = res256[:, C:]
    cnt_b = m_b.sum(axis=1) / (C - 1)
    conf_b = cnt_b[:, None] - m_b
    return conf_a + conf_b


def f1_from_conf(conf_f):
    """Replicates the reference's (quirky) F1 reduction on a [128,128] f32
    confusion matrix."""
    conf_f = conf_f.astype(np.float32)
    TP = np.diagonal(conf_f).astype(np.float32)
    FP = np.float32(C - 1) * conf_f[:, 1] + conf_f[:, 0]
    FN = np.float32(C - 1) * conf_f[1, :] + conf_f[0, :]
    sensitivity = TP / (TP + FN + EPS)
    precision = TP / (TP + FP + EPS)
    f1 = np.float32(2.0) * (precision * sensitivity / (precision + sensitivity + EPS))
    return np.array(np.mean(f1), dtype=np.float32)


def make_in_maps(y_pred, y_true, cfg=None):
    """Shard + lay out the full inputs for the 8-core SPMD program."""
    if cfg is None:
        cfg = BEST
    iota_np = _iota_np()
    if cfg.get("xdt", F32) == BF16:
        # stream the scores at bf16: halves HBM traffic; argmax ties from
        # the rounding shift the conf counts by ~0.4% of rows (measured F1
        # rel err 1.0e-3, tolerance 2e-2)
        y_pred = y_pred.astype(ml_dtypes.bfloat16)
    yp_sh = y_pred.reshape(N_CORES, SHARD, C)
    yt_sh = y_true.reshape(N_CORES, SHARD)
    rpp = cfg["rpp"]
    extra = {}
    if cfg.get("dve_mask") == "pemxr":
        extra["ident"] = np.eye(C, dtype=ml_dtypes.bfloat16)
        sel = np.zeros((rpp, rpp * 32), dtype=ml_dtypes.bfloat16)
        for q in range(rpp):
            sel[q, q * 32:(q + 1) * 32] = 1.0
        extra["sel"] = sel
    return [{
        "yp": yp_sh[i],
        "yt": _host_layout_scatter_idx(yt_sh[i], rpp),
        "iota": iota_np,
        **extra,
    } for i in range(N_CORES)]


def kernel(y_pred, y_true, _spmd_runner=None, **_ignored):
    y_pred = np.ascontiguousarray(np.asarray(y_pred), dtype=np.float32)
    y_true = np.asarray(y_true)
    assert y_pred.shape == (N_TOTAL, C)

    nc = _get_program(**BEST)
    in_maps = make_in_maps(y_pred, y_true)
    runner = _spmd_runner or bass_utils.run_bass_kernel_spmd
    res = runner(nc, in_maps, core_ids=list(range(N_CORES)))
    results = res.results if hasattr(res, "results") else res
    conf = np.zeros((128, C), dtype=np.float64)
    for r in results:
        conf += conf_from_banks(r["conf"])
    return f1_from_conf(conf.astype(np.float32))

